# revision 6
# baseline (speedup 1.0000x reference)
"""Trainium2 Bass kernel for the Clifford (geometric) product on Cl(3,0), v2.

Strategy (per NeuronCore, batch sharded 8 ways):
  Cl(3,0) ~= Mat2(C) via the Pauli representation. The product becomes a
  2x2 complex matrix multiply: 32 real multiplies + 48 adds per sample,
  vs 64 multiplies + 56 adds for the direct structure-constant form.

  Layout: per-partition planar bf16. ScalarE deinterleaves the DMA'd
  fp32 interleaved tiles into bf16 component planes (folding the 1/2
  scale of the forward transform into the cast). All DVE tensor_tensor
  ops then run on step-1 bf16 operands -> 2x_1P perf mode (2 elem/cyc).
  GPSIMD takes the B-transform and L2 stages. The inverse transform
  writes fp32 interleaved output directly (1x, small), so the output DMA
  is a plain HWDGE fp32 store.

  Plane orders:
    A[p,r,s]: idx 4p+2r+s (s=0:Re,1:Im) of matrix entry M[p][r]
    B[r,q,s]: idx 4r+2q+s of N[r][q]
    pc slot = 16p + 8q + 4*(sM^sN) + 2*sM + r   (products M[p,r]*N[r,q])
    c[o]: idx 4p+2q+ri of C[p][q] (ri=0:Re,1:Im)
"""

import os

os.environ.setdefault("BY_DEFAULT_DISABLE_SUBTILE_DEPS", "1")

import numpy as np

N_TOTAL = 4194304
N_CORES = 8
NC = N_TOTAL // N_CORES
P = 128

F32 = None
BF16 = None


def _dt():
    global F32, BF16
    import concourse.mybir as mybir

    F32 = mybir.dt.float32
    BF16 = mybir.dt.bfloat16


# --------------------------------------------------------------- op tables
# spec = (tile, offset_elems, dims) with dims [(stride, count), ...] in elems,
# given e = samples per plane. Tiles: a32i,b32i,o32i fp32 [128, 8e];
# a16p,b16p,A,B,c bf16 [128, 8e]; pc bf16 [128, 32e].


def gen_ops(e, split_q=False, l1_split=8, tfA_src="a32i", inv_to_pc=False):
    """Return list of (group, alu, out_spec, in0_spec, in1_spec).

    c-planes live in pc slots 4*o+1 (free after L1, which only writes ranks
    0/2); the inverse transform writes bf16 planar o16p, re-interleaved to
    fp32 by ScalarE. l1_split: number of L1 bases (of 8) in the first op
    (group L1a); the rest go in L1b (assignable to another engine).
    tfA_src="a32i": the A-transform reads the fp32 interleaved tile
    directly (component c at elem offset c, sample stride 8) — no a16p
    deinterleave; the 1/2 scale folds into the b deinterleave instead.
    """
    ops = []

    def pl(t, off, dims):  # offsets/strides in plane units -> elems
        return (t, off * e, [(s * e, c) for (s, c) in dims[:-1]] + [dims[-1]])

    def il(t, off, dims):  # interleaved: comp offset/strides raw, samples x8
        return (t, off, [(s, c) for (s, c) in dims[:-1]] + [(8, dims[-1][1])])

    # transforms: (out two planes), (in0 planes), (in1 planes)
    tr = [
        ("add", (0, 1), (0, 4), (3, 4)),
        ("add", (4, 1), (1, 1), (5, 1)),
        ("sub", (2, 1), (1, 5), (5, -3)),
        ("sub", (6, 1), (0, 7), (3, 1)),
    ]
    srcA = (tfA_src, il) if tfA_src == "a32i" else (tfA_src, pl)
    for dst, (src, f) in (("A", srcA), ("B", ("b16p", pl))):
        for alu, (oo, od), (i0, d0), (i1, d1) in tr:
            ops.append((
                f"tf{dst}", alu,
                pl(dst, oo, [(od, 2), (1, e)]),
                f(src, i0, [(d0, 2), (1, e)]),
                f(src, i1, [(d1, 2), (1, e)]),
            ))

    # products: (p, sM, sN) over (q, r) grid
    for p in range(2):
        for sM in range(2):
            for sN in range(2):
                oo = 16 * p + 4 * (sM ^ sN) + 2 * sM
                if split_q:
                    for q in range(2):
                        ops.append((
                            "prod", "mult",
                            pl("pc", oo + 8 * q, [(1, 2), (1, e)]),
                            pl("A", 4 * p + sM, [(2, 2), (1, e)]),
                            pl("B", sN + 2 * q, [(4, 2), (1, e)]),
                        ))
                else:
                    ops.append((
                        "prod", "mult",
                        pl("pc", oo, [(8, 2), (1, 2), (1, e)]),
                        pl("A", 4 * p + sM, [(0, 2), (2, 2), (1, e)]),
                        pl("B", sN, [(2, 2), (4, 2), (1, e)]),
                    ))

    # L1: slots base+2t += base+2t+1 over 8 bases, split l1_split/(8-l1_split)
    for grp, b0, nb in (("L1a", 0, l1_split), ("L1b", l1_split, 8 - l1_split)):
        if nb <= 0:
            continue
        ops.append((
            grp, "add",
            pl("pc", 4 * b0, [(4, nb), (2, 2), (1, e)]),
            pl("pc", 4 * b0, [(4, nb), (2, 2), (1, e)]),
            pl("pc", 4 * b0 + 1, [(4, nb), (2, 2), (1, e)]),
        ))

    # L2 into c-planes at pc slot 4*o+1: c[Re:o=2(2p+q)] = rank0 - rank2 ;
    # c[Im] = rank0(+4) + rank2(+6)
    # L2a (DVE) emitted before L2s (GPSIMD): whole-tile dep tracking
    # serializes pc ops in emission order, so this order keeps the DVE from
    # waiting on the GPSIMD op; with stagger_inv the inverse transform runs
    # a tile later and never waits on L2s either.
    ops.append((
        "L2a", "add",
        pl("pc", 5, [(16, 2), (8, 2), (1, e)]),
        pl("pc", 4, [(16, 2), (8, 2), (1, e)]),
        pl("pc", 6, [(16, 2), (8, 2), (1, e)]),
    ))
    ops.append((
        "L2s", "sub",
        pl("pc", 1, [(16, 2), (8, 2), (1, e)]),
        pl("pc", 0, [(16, 2), (8, 2), (1, e)]),
        pl("pc", 2, [(16, 2), (8, 2), (1, e)]),
    ))

    # inverse: bf16 planar o16p; c-plane o at pc slot 4*o+1
    inv = [
        ("add", (0, 7), (0, 1), (6, 1)),
        ("add", (1, 5), (2, 1), (4, 1)),
        ("sub", (3, 1), (0, 1), (6, 1)),
        ("sub", (5, -3), (4, 1), (2, 1)),
    ]
    for alu, (oo, od), (i0, d0), (i1, d1) in inv:
        if inv_to_pc:
            out = pl("pc", 4 * oo + 3, [(4 * od, 2), (1, e)])
        else:
            out = pl("o16p", oo, [(od, 2), (1, e)])
        ops.append((
            "inv", alu,
            out,
            pl("pc", 4 * i0 + 1, [(4 * d0, 2), (1, e)]),
            pl("pc", 4 * i1 + 1, [(4 * d1, 2), (1, e)]),
        ))
    return ops


# ------------------------------------------------------- numpy validation
def _walk(buf, spec):
    t, off, dims = spec
    idx = np.zeros([c for (_, c) in dims], dtype=np.int64) + off
    for d, (s, c) in enumerate(dims):
        sh = [1] * len(dims)
        sh[d] = c
        idx = idx + (np.arange(c) * s).reshape(sh)
    return buf[t], idx


def simulate(a, b, e, **kw):
    """Run the op tables in numpy (fp32, per partition-lane) for validation."""
    n = a.shape[0]
    assert n == e
    bufs = {
        "a32i": (a.reshape(-1) * 1.0).astype(np.float32),
        "b32i": b.reshape(-1).astype(np.float32),
        "a16p": np.zeros(8 * e, np.float32),
        "b16p": np.zeros(8 * e, np.float32),
        "A": np.zeros(8 * e, np.float32),
        "B": np.zeros(8 * e, np.float32),
        "pc": np.zeros(32 * e, np.float32),
        "o16p": np.zeros(8 * e, np.float32),
        "o32i": np.zeros(8 * e, np.float32),
    }
    # deinterleave (+0.5 scale on b; a read raw by tfA from a32i)
    for c_ in range(8):
        bufs["a16p"][c_ * e:(c_ + 1) * e] = bufs["a32i"][c_::8]
        bufs["b16p"][c_ * e:(c_ + 1) * e] = 0.5 * bufs["b32i"][c_::8]
    alu = {"add": np.add, "sub": np.subtract, "mult": np.multiply}
    for (_, op, o, i0, i1) in gen_ops(e, **kw):
        ob, oi = _walk(bufs, o)
        b0, x0 = _walk(bufs, i0)
        b1, x1 = _walk(bufs, i1)
        ob[oi] = alu[op](b0[x0], b1[x1])
    # re-interleave
    for c_ in range(8):
        if kw.get("inv_to_pc"):
            bufs["o32i"][c_::8] = bufs["pc"][(4 * c_ + 3) * e:(4 * c_ + 3) * e + e]
        else:
            bufs["o32i"][c_::8] = bufs["o16p"][c_ * e:(c_ + 1) * e]
    return bufs["o32i"].reshape(e, 8)


# ------------------------------------------------------------ bass builder
def _mkap(base, dims, offset):
    import concourse.mybir as mybir

    ap = base.copy()
    part = list(base.ap[0])
    ap.ap = mybir.VecI64Pair([part] + [[d, c] for (d, c) in dims])
    ap.offset = base.offset + offset
    return ap


# op-group -> engine name ("vector" | "gpsimd"); "deint" also allows "scalar"
DEFAULT_ASSIGN = {
    "deint": "scalar",
    "tfA": "vector",
    "tfB": "vector",
    "prod": "vector",
    "L1a": "vector",
    "L1b": "vector",
    "L2s": "gpsimd",
    "L2a": "vector",
    "inv": "vector",
}


def build_nc(nc_mv=NC, e=256, assign=None, split_q=False, l1_split=8,
             tfA_src="a16p", reint_gps=0, nbufs=3, nbufs_io=None,
             warm=(64, 192), inv_to_pc=False, sb_tags=(), dma_scratch=16384,
             bufs_map=None, stagger_inv=False, reint_lag=2):
    import concourse.bacc as bacc
    import concourse.mybir as mybir
    from concourse.tile import TileContext

    _dt()
    assign = dict(DEFAULT_ASSIGN, **(assign or {}))
    n_tiles = nc_mv // (P * e)
    assert n_tiles * P * e == nc_mv

    # Tile schedule: optionally split the first and last full tile so the
    # pipeline fills (and drains) in a fraction of a full-tile time.
    if warm and n_tiles >= 2:
        tile_es = list(warm) + [e] * (n_tiles - 2) + list(warm)[::-1]
        assert sum(warm) == e
    else:
        tile_es = [e] * n_tiles
    ops_by_e = {
        et: gen_ops(et, split_q=split_q, l1_split=l1_split, tfA_src=tfA_src,
                    inv_to_pc=inv_to_pc)
        for et in set(tile_es)
    }

    nc = bacc.Bacc("TRN2", target_bir_lowering=False, debug=False,
                   dynamic_dma_scratch_size=dma_scratch)
    a_d = nc.dram_tensor("a", [nc_mv, 8], F32, kind="ExternalInput")
    b_d = nc.dram_tensor("b", [nc_mv, 8], F32, kind="ExternalInput")
    o_d = nc.dram_tensor("o", [nc_mv, 8], F32, kind="ExternalOutput")

    def views(t_s0, et):
        r0, r1 = t_s0, t_s0 + P * et
        va = a_d.ap()[r0:r1].rearrange("(p e) c -> p (e c)", p=P)
        vb = b_d.ap()[r0:r1].rearrange("(p e) c -> p (e c)", p=P)
        vo = o_d.ap()[r0:r1].rearrange("(p e) c -> p (e c)", p=P)
        return va, vb, vo

    ALU = {
        "add": mybir.AluOpType.add,
        "sub": mybir.AluOpType.subtract,
        "mult": mybir.AluOpType.mult,
    }

    with TileContext(nc) as tc:
        with (
            tc.tile_pool(name="io", bufs=nbufs_io or nbufs) as io_pool,
            tc.tile_pool(name="pln", bufs=nbufs) as pln_pool,
        ):
            def emit_reint(src_t, o32i, vo, et):
                # re-interleave + cast to fp32 on ScalarE. src_t is o16p
                # (plane stride et) or pc (plane = slot 4c+3, stride 4*et).
                if inv_to_pc:
                    sview = _mkap(src_t, [(4 * et, 8), (1, et)], 3 * et)
                else:
                    sview = _mkap(src_t, [(et, 8), (1, et)], 0)
                nc.scalar.copy(_mkap(o32i, [(1, 8), (8, et)], 0), sview)
                nc.sync.dma_start(out=vo, in_=_mkap(o32i, [(1, 8 * et)], 0))

            pending = None if stagger_inv else []
            s0 = 0
            for t, et in enumerate(tile_es):
                va, vb, vo = views(s0, et)
                s0 += P * et
                def mk(pool, tag, w, dt):
                    kw = {"tag": tag, "name": tag}
                    nb = (bufs_map or {}).get(tag, 1 if tag in sb_tags else None)
                    if nb:
                        kw["bufs"] = nb
                    return pool.tile([P, w], dt, **kw)

                a32i = mk(io_pool, "a32i", 8 * e, F32)
                b32i = mk(io_pool, "b32i", 8 * e, F32)
                o32i = mk(io_pool, "o32i", 8 * e, F32)
                b16p = mk(pln_pool, "b16p", 8 * e, BF16)
                A_t = mk(pln_pool, "A", 8 * e, BF16)
                B_t = mk(pln_pool, "B", 8 * e, BF16)
                pc_t = mk(pln_pool, "pc", 32 * e, BF16)

                tiles = {
                    "a32i": a32i, "b32i": b32i, "o32i": o32i,
                    "b16p": b16p, "A": A_t, "B": B_t, "pc": pc_t,
                }
                if not inv_to_pc:
                    tiles["o16p"] = mk(pln_pool, "o16p", 8 * e, BF16)
                if tfA_src == "a16p":
                    a16p = pln_pool.tile([P, 8 * e], BF16, tag="a16p")
                    tiles["a16p"] = a16p

                nc.sync.dma_start(out=_mkap(a32i, [(1, 8 * et)], 0), in_=va)
                nc.scalar.dma_start(out=_mkap(b32i, [(1, 8 * et)], 0), in_=vb)

                # deinterleave + cast; the 1/2 transform scale folds into b.
                if tfA_src == "a16p":
                    nc.scalar.copy(
                        _mkap(tiles["a16p"], [(et, 8), (1, et)], 0),
                        _mkap(a32i, [(1, 8), (8, et)], 0),
                    )
                nc.scalar.mul(
                    _mkap(b16p, [(et, 8), (1, et)], 0),
                    _mkap(b32i, [(1, 8), (8, et)], 0),
                    0.5,
                )

                def emit_ops(op_list, tl):
                    for (grp, op, o, i0, i1) in op_list:
                        eng = nc.vector if assign[grp] == "vector" else nc.gpsimd
                        to, oo, od = o
                        t0, f0, d0 = i0
                        t1, f1, d1 = i1
                        eng.tensor_tensor(
                            out=_mkap(tl[to], od, oo),
                            in0=_mkap(tl[t0], d0, f0),
                            in1=_mkap(tl[t1], d1, f1),
                            op=ALU[op],
                        )

                all_ops = ops_by_e[et]
                if stagger_inv:
                    main_ops = [x for x in all_ops if x[0] != "inv"]
                    inv_ops = [x for x in all_ops if x[0] == "inv"]
                    # previous tile's inverse transform runs first in this
                    # tile's DVE block: it has no ScalarE dependency, so it
                    # absorbs any lateness of this tile's deinterleaves.
                    if pending is not None:
                        emit_ops(pending[0], pending[1])
                    emit_ops(main_ops, tiles)
                    if pending is not None:
                        emit_reint(*pending[2])
                    src_t = tiles["pc"] if inv_to_pc else tiles["o16p"]
                    pending = (inv_ops, tiles, (src_t, o32i, vo, et))
                else:
                    emit_ops(all_ops, tiles)
                    # stagger: emit the re-interleave+store of the tile
                    # `reint_lag` back, so ScalarE's deinterleaves never
                    # queue behind a reint that waits on a recent DVE op.
                    src_t = tiles["pc"] if inv_to_pc else tiles["o16p"]
                    pending.append((src_t, o32i, vo, et))
                    if len(pending) > reint_lag:
                        emit_reint(*pending.pop(0))
            if stagger_inv:
                emit_ops(pending[0], pending[1])
                emit_reint(*pending[2])
            else:
                for p in pending:
                    emit_reint(*p)
    nc.compile()
    return nc


_NC_CACHE = {}


def _get_nc(nc_mv, e=256, **kw):
    key = (nc_mv, e, tuple(sorted(kw.items())))
    if key not in _NC_CACHE:
        _NC_CACHE[key] = build_nc(nc_mv, e, **kw)
    return _NC_CACHE[key]


def kernel(a, b, M=None, **_):
    from concourse import bass_utils

    a = np.ascontiguousarray(np.asarray(a, dtype=np.float32))
    b = np.ascontiguousarray(np.asarray(b, dtype=np.float32))
    n = a.shape[0]
    assert n % N_CORES == 0
    nc_mv = n // N_CORES
    nc = _get_nc(nc_mv)
    a_sh = a.reshape(N_CORES, nc_mv, 8)
    b_sh = b.reshape(N_CORES, nc_mv, 8)
    in_maps = [{"a": a_sh[c], "b": b_sh[c]} for c in range(N_CORES)]
    res = bass_utils.run_bass_kernel_spmd(nc, in_maps, core_ids=list(range(N_CORES)))
    out = np.concatenate([r["o"].reshape(nc_mv, 8) for r in res.results], axis=0)
    return out
